# revision 36
# baseline (speedup 1.0000x reference)
"""MoE routing kernel for Trainium2 (8 NeuronCores, data-parallel over batch).

Reference computation (B=1024, PHASE=64, GATE=128, K=8, D=512):
    coeff = softmax(gateMLP(phase))                       # [B, K]
    per layer l in 0..2:
        y = sum_k coeff[:,k] * (y @ W_l[k]) + coeff @ b_l # [B, D]
        y = elu(y)  (layers 0,1 only)

Device mapping (per core, B_local = 128 rows). The schedule is built
around the measured DMA fabric: 16 engines x ~25 GB/s = ~400 GB/s
aggregate; packets are served FIFO per queue and round-robin BY PACKET
across queues (so small-row blobs starve behind 4KB weight rows), and
the Scalar HWDGE adds ~1.5-2.4us descriptor->packet latency vs ~0.15us
on the Sync HWDGE.  Hence ALL loads go on the Sync queue, consts first:
  - one merged fp16 const blob (gate weights + biases, ph.T packed into
    the unused partitions below gw0) then x.T, then W0[0..7], the bias
    banks, W1, W2; the last expert chunk (2,7) is split per-IC so its
    matmuls start per 128KB sub-chunk.  ~12.9 MB total => ~32us stream.
  - Activations carry a +1 shift: y' = elu(y)+1, shift absorbed into the
    next layer's bias (b' = b - W.sum(axis=in)) on the host.  ELU is
    r=relu(x) on DVE, e=exp(x) on ACT, out=min(e,1)+r on DVE.  Engines
    reading the same PSUM bank serialize, so boundary ELUs are split in
    column halves (reads r0,e0,r1,e1 interleave; z matmuls for the first
    half start while the second half drains).
  - Softmax is normalized right after the gate (e_hat = e/sum(e)); all
    PSUM drains are scale-free.
  - Layer 0 runs gate-independent per-expert matmuls (x.T @ W0[k], paced
    by W-chunk arrival), hand-interleaved with the gate's small matmuls,
    then post-scales with diag(e_hat) + bias matmul.
  - Layers 1-2 pre-scale: z_k.T = y.T * e_hat[:,k] via y_chunk.T @
    diag(e_hat_k), 4 experts' diagonals per N=512 fp16 matmul; then 32
    accumulating matmuls + one bias matmul per layer into one PSUM bank.
    The z transposes share the 4-bank PSUM pool with layer-0's experts
    (free by then) and drain on three engines (DVE/ACT/GpSimd) so the
    2-bank ping-pong never gates the PE.
  - Final drain is split in column halves across ACT/DVE with two output
    DMAs so the tail pipeline overlaps.
"""

import numpy as np

import concourse.mybir as mybir
import concourse.tile as tile
from concourse import bacc
from concourse.masks import make_identity

AFT = mybir.ActivationFunctionType
ALU = mybir.AluOpType
F32 = mybir.dt.float32
F16 = mybir.dt.float16
AX = mybir.AxisListType

B, PHASE, GATE, K, D = 1024, 64, 128, 8, 512
NCORES = 8
BL = B // NCORES          # 128 rows per core
IC = D // 128             # 4 contraction chunks of 128
LW = K * IC * D           # weight columns per layer (16384)


def emit_moe(tc, out_ap, ins):
    """Emit the per-core MoE program. ins is a dict of DRAM APs."""
    nc = tc.nc

    with (
        tc.tile_pool(name="consts", bufs=1) as cpool,
        tc.tile_pool(name="ypool", bufs=2) as ypool,
        tc.tile_pool(name="zpool", bufs=2) as zpool,
        tc.tile_pool(name="tmp", bufs=3) as tpool,
        tc.tile_pool(name="ps_out", bufs=2, space="PSUM") as ps_out,
        tc.tile_pool(name="ps_z", bufs=2, space="PSUM") as ps_z,
        tc.tile_pool(name="ps_exp", bufs=4, space="PSUM") as ps_exp,
    ):
        # ACT warmup: pull the Exp activation table off the critical path.
        t_ones = cpool.tile([1, GATE], F16)
        nc.vector.memset(t_ones, 1.0)
        t_warm = tpool.tile([1, 8], F32, tag="warm")
        nc.scalar.activation(t_warm, t_ones[:, :8], AFT.Exp)

        # identities built on-chip (gpsimd is idle; saves DMA bytes)
        t_ident = cpool.tile([128, 128], F16)
        make_identity(nc, t_ident)
        t_ident32 = cpool.tile([128, 128], F32)
        make_identity(nc, t_ident32)

        # ---- all consts in ONE descriptor, FIRST on the Sync HWDGE queue
        # (few small descriptors serialize the DGE ring ramp) ------------
        # gc [128, 928] f16:
        #   0:128 gw1 | 128:136 gw2 | 136:137 gb1a | row0 137:145 gb2
        #   160:288 rows 0:64 gw0 | 288:416 rows 0:64 ph.T | 416:928 x
        t_gc = cpool.tile([128, 928], F16)
        nc.sync.dma_start(out=t_gc, in_=ins["gc"])
        t_gw1 = t_gc[:, 0:128]
        t_gw2 = t_gc[:, 128:136]
        t_gb2 = t_gc[0:1, 137:145]
        t_gw0 = t_gc[0:PHASE, 160:288]
        t_phT = t_gc[0:PHASE, 288:416]
        t_xT = t_gc[:, 416:928]
        # scalar APs must be f32: cast the f16 gb1 column on-chip
        t_gb1 = cpool.tile([128, 1], F32)
        nc.vector.tensor_copy(out=t_gb1, in_=t_gc[:, 136:137])

        # ---- expert weights on the same queue: bias banks first (needed
        # by the early-interleaved combine bias matmul), then W0 (consumed
        # while streaming), W1, W2; the final chunk (2,7) split per-IC so
        # its matmuls start per 128KB sub-chunk --------------------------
        t_w = cpool.tile([128, 3 * LW], F16)
        t_bias = cpool.tile([K, 3 * D], F16)
        nc.sync.dma_start(out=t_bias, in_=ins["eb"])

        def w_dma(l, k):
            nc.sync.dma_start(
                out=t_w[:, l * LW + k * 2048:l * LW + (k + 1) * 2048],
                in_=ins["W"][l, k],
            )

        for k in range(K):
            w_dma(0, k)
        for k in range(K):
            w_dma(1, k)
        for k in range(K - 1):
            w_dma(2, k)
        for ic in range(IC):
            nc.sync.dma_start(
                out=t_w[:, 2 * LW + (K - 1) * 2048 + ic * 512:
                        2 * LW + (K - 1) * 2048 + (ic + 1) * 512],
                in_=ins["W"][2, K - 1][:, ic * 512:(ic + 1) * 512],
            )

        # ---- gate: pure latency chain, done ~15us ----------------------
        t_allones = cpool.tile([128, 128], F16)
        nc.vector.memset(t_allones, 1.0)

        p_g = ps_z.tile([128, 512], F32, tag="zps")
        nc.tensor.matmul(p_g[:GATE, :BL], lhsT=t_gw0, rhs=t_phT, start=True, stop=True)
        h1 = tpool.tile([GATE, BL], F16, tag="h")
        _elu1(nc, tpool, h1, p_g[:GATE, :BL], bias=0.0)

        p_g2 = ps_z.tile([128, 512], F32, tag="zps")
        nc.tensor.matmul(p_g2[:GATE, :BL], lhsT=t_gw1, rhs=h1, start=True, stop=True)
        h2 = tpool.tile([GATE, BL], F16, tag="h")
        _elu1(nc, tpool, h2, p_g2[:GATE, :BL], bias=t_gb1)

        # logits[b, k] (normal layout; gb2 via ones-row matmul)
        p_lg = ps_z.tile([128, 512], F32, tag="zps")
        nc.tensor.matmul(p_lg[:BL, :K], lhsT=h2, rhs=t_gw2, start=True, stop=False)
        nc.tensor.matmul(p_lg[:BL, :K], lhsT=t_ones, rhs=t_gb2, start=False, stop=True)

        # e_hat = softmax(logits), normalized immediately (cheap [128,8] ops)
        t_nmx = tpool.tile([BL, 1], F32)
        nc.vector.reduce_max(t_nmx, p_lg[:BL, :K], axis=AX.X, negate=True)
        t_e = cpool.tile([BL, K], F32)
        nc.scalar.activation(t_e, p_lg[:BL, :K], AFT.Exp, bias=t_nmx, scale=1.0)
        t_sum = tpool.tile([BL, 1], F32)
        nc.vector.reduce_sum(t_sum, t_e, axis=AX.X)
        t_rcp = tpool.tile([BL, 1], F32)
        nc.vector.reciprocal(t_rcp, t_sum)
        t_eh = cpool.tile([BL, K], F32)
        nc.vector.tensor_scalar_mul(t_eh, t_e, t_rcp)

        # e_hat.T (fp16) for the mixed-bias matmuls
        p_et = ps_z.tile([128, 512], F32, tag="zps")
        nc.tensor.transpose(p_et[:K, :BL], t_eh, t_ident32)
        t_eT = cpool.tile([K, BL], F16)
        nc.scalar.copy(t_eT, p_et[:K, :BL])

        # diag quads: [diag(eh_{4q}) .. diag(eh_{4q+3})], split DVE/ACT
        t_diag = cpool.tile([128, 2 * 512], F16)
        for k in range(K):
            dst = t_diag[:, k * 128:(k + 1) * 128]
            sc = t_eh[:, k:k + 1]
            if k % 2 == 0:
                nc.vector.tensor_scalar_mul(dst, t_ident, sc)
            else:
                nc.scalar.activation(dst, t_ident, AFT.Copy, scale=sc)

        # ---- layer-0 z-tiles WITHOUT PSUM: es panels (e_hat replicated
        # across partitions) via one all-ones matmul per quad, then 32
        # small f16 multiplies on DVE/GpSimd build zx = x.T * eh directly
        # in SBUF by ~17us — so layer 0 is a single arrival-paced
        # accumulation with NO drains or combine trailing its last chunk.
        t_es = cpool.tile([128, 1024], F16)
        for q in range(2):
            p_es = ps_exp.tile([128, 512], F32, tag="pexp")
            nc.tensor.matmul(
                p_es, lhsT=t_allones, rhs=t_diag[:, q * 512:(q + 1) * 512],
                start=True, stop=True,
            )
            dst = t_es[:, q * 512:(q + 1) * 512]
            if q == 0:
                nc.vector.tensor_copy(out=dst, in_=p_es)
            else:
                nc.scalar.copy(dst, p_es)

        t_zx = zpool.tile([128, K * D], F16, tag="zx")
        for k in range(K):
            q, kq = divmod(k, 4)
            for ic in range(IC):
                dst = t_zx[:, q * 2048 + ic * 512 + kq * 128:
                           q * 2048 + ic * 512 + (kq + 1) * 128]
                eng = nc.vector if (k * IC + ic) % 2 == 0 else nc.gpsimd
                eng.tensor_tensor(
                    out=dst,
                    in0=t_xT[:, ic * 128:(ic + 1) * 128],
                    in1=t_es[:, q * 512 + kq * 128:q * 512 + (kq + 1) * 128],
                    op=ALU.mult,
                )

        # ---- all 3 layers: bias + 32 accumulating matmuls into one PSUM
        # bank, k-major so consumption tracks chunk arrival.  Layers 1-2
        # first pre-scale: z_k.T = y.T * eh[:,k] via y_chunk.T @ diag
        # quads; z PSUM rotates through 6 banks (4 ps_exp + the 2 ps_z
        # banks idle after the gate); drains alternate DVE/ACT. ----------
        t_zl = t_zx
        for l in range(3):
            if l > 0:
                t_zl = zpool.tile([128, K * D], F16, tag="z")
                zi = 0
                for q in range(2):
                    for ic in range(IC):
                        zpl = ps_z if zi % 4 == 3 else ps_exp
                        p_z = zpl.tile([128, 512], F32,
                                       tag="zps" if zpl is ps_z else "pexp")
                        nc.tensor.matmul(
                            p_z,
                            lhsT=y[:, ic * 128:(ic + 1) * 128],
                            rhs=t_diag[:, q * 512:(q + 1) * 512],
                            start=True,
                            stop=True,
                        )
                        dst = t_zl[:, q * 2048 + ic * 512:q * 2048 + (ic + 1) * 512]
                        if zi % 2 == 0:
                            nc.vector.tensor_copy(out=dst, in_=p_z)
                        else:
                            nc.scalar.copy(dst, p_z)
                        zi += 1

            # step B: out = eh @ b_l + sum_k z_k @ W_l[k] (bias first, so the
            # group's last matmul is the last expert and the drain starts
            # right after the last weight chunk is consumed)
            p_o = ps_out.tile([BL, D], F32, tag="out")
            nc.tensor.matmul(
                p_o,
                lhsT=t_eT,
                rhs=t_bias[:, l * D:(l + 1) * D],
                start=True,
                stop=False,
            )
            for k in range(K):
                q, kq = divmod(k, 4)
                for ic in range(IC):
                    nc.tensor.matmul(
                        p_o,
                        lhsT=t_zl[:, q * 2048 + ic * 512 + kq * 128:
                                  q * 2048 + ic * 512 + (kq + 1) * 128],
                        rhs=t_w[:, l * LW + k * 2048 + ic * 512:
                                l * LW + k * 2048 + (ic + 1) * 512],
                        start=False,
                        stop=(k == K - 1 and ic == IC - 1),
                    )

            if l < 2:
                y_next = ypool.tile([BL, D], F16, tag="y")
                _elu_split(nc, tpool, y_next, p_o)
                y = y_next
            else:
                # drain in column halves on two engines, two output DMAs
                t_out = ypool.tile([BL, D], F32, tag="yout")
                nc.scalar.copy(t_out[:, 0:256], p_o[:, 0:256])
                nc.sync.dma_start(out=out_ap[:, 0:256], in_=t_out[:, 0:256])
                nc.vector.tensor_copy(out=t_out[:, 256:512], in_=p_o[:, 256:512])
                nc.sync.dma_start(out=out_ap[:, 256:512], in_=t_out[:, 256:512])


def _elu1(nc, tpool, out, pre, bias):
    """out = elu(x)+1 = relu(x) + min(exp(x), 1); x = pre + bias.
    Small (gate) version: single-shot."""
    shape = [pre.partition_size(), pre.free_size()]
    t_r = tpool.tile(shape, F16, tag="elu_r")
    nc.vector.tensor_scalar(t_r, pre, bias, 0.0, op0=ALU.add, op1=ALU.max)
    t_e = tpool.tile(shape, F32, tag="elu_e")
    nc.scalar.activation(t_e, pre, AFT.Exp, bias=bias, scale=1.0)
    nc.vector.scalar_tensor_tensor(
        out, in0=t_e, scalar=1.0, in1=t_r, op0=ALU.min, op1=ALU.add
    )


def _elu_split(nc, tpool, out, pre):
    """Column-split elu+1 on a [128, 512] PSUM bank.  PSUM readers
    serialize, so reads go r0, e0, r1, e1 (each half-size); the combine
    for the first half runs while the second half is still being read,
    and downstream matmuls on columns 0:256 can start ~1us earlier."""
    H = pre.free_size() // 2
    P = pre.partition_size()
    rs, es = [], []
    for h in range(2):
        cols = slice(h * H, (h + 1) * H)
        t_r = tpool.tile([P, H], F16, tag=f"elu_r{h}")
        nc.vector.tensor_scalar(t_r, pre[:, cols], 0.0, 0.0, op0=ALU.add, op1=ALU.max)
        t_e = tpool.tile([P, H], F32, tag=f"elu_e{h}")
        nc.scalar.activation(t_e, pre[:, cols], AFT.Exp)
        rs.append(t_r)
        es.append(t_e)
    for h in range(2):
        cols = slice(h * H, (h + 1) * H)
        nc.vector.scalar_tensor_tensor(
            out[:, cols], in0=es[h], scalar=1.0, in1=rs[h], op0=ALU.min, op1=ALU.add
        )


def _prep_host(x, phase, gw0, gb0, gw1, gb1, gw2, gb2, W0, b0, W1, b1, W2, b2):
    """Host-side packing. Returns per-core input maps."""
    f32 = np.float32
    f16 = np.float16

    # weights blob: [3, 8, 128, 2048]; [l, k, p, ic*512 + o] = W_l[k, ic*128+p, o]
    W = np.stack([W0, W1, W2]).astype(f32)  # [3, 8, 512, 512]
    Wb = (
        W.reshape(3, K, IC, 128, D)
        .transpose(0, 1, 3, 2, 4)
        .reshape(3, K, 128, IC * D)
        .astype(f16)
    )
    # +1-shift corrections: layer l>0 consumes y'+1, gate layers 1,2 consume h'+1
    b0a = np.asarray(b0, f32)
    b1a = np.asarray(b1, f32) - np.asarray(W1, f32).sum(axis=1)
    b2a = np.asarray(b2, f32) - np.asarray(W2, f32).sum(axis=1)
    eb = np.concatenate([b0a, b1a, b2a], axis=1).astype(f16)  # [8, 1536]
    gb1a = np.asarray(gb1, f32) - np.asarray(gw1, f32).sum(axis=0)
    gb2a = np.asarray(gb2, f32) - np.asarray(gw2, f32).sum(axis=0)

    # gc blob [128, 928] f16 (layout documented in emit_moe)
    gc = np.zeros((128, 928), f16)
    gc[:, 0:128] = np.asarray(gw1, f32).astype(f16)
    gc[:, 128:136] = np.asarray(gw2, f32).astype(f16)
    gc[:, 136] = gb1a.astype(f16)
    gc[0, 137:145] = gb2a.astype(f16)
    gc[0:PHASE, 160:288] = np.asarray(gw0, f32).astype(f16)

    per_core = []
    for c in range(NCORES):
        sl = slice(c * BL, (c + 1) * BL)
        gcc = gc.copy()
        gcc[0:PHASE, 288:416] = np.asarray(phase[sl], f32).T.astype(f16)
        xs = np.asarray(x[sl]).astype(f16)
        gcc[:, 416:928] = (
            xs.T.reshape(IC, 128, BL).transpose(1, 0, 2).reshape(128, IC * BL)
        )
        per_core.append(
            {
                "gc": np.ascontiguousarray(gcc),
                "eb": eb,
                "W": Wb,
            }
        )
    return per_core


def _declare_dram(nc):
    ins = {
        "gc": nc.dram_tensor("gc", [128, 928], F16, kind="ExternalInput").ap(),
        "eb": nc.dram_tensor("eb", [K, 3 * D], F16, kind="ExternalInput").ap(),
        "W": nc.dram_tensor("W", [3, K, 128, IC * D], F16, kind="ExternalInput").ap(),
    }
    out = nc.dram_tensor("out", [BL, D], mybir.dt.float32, kind="ExternalOutput").ap()
    return ins, out


_CACHED = None


def _build():
    global _CACHED
    if _CACHED is None:
        nc = bacc.Bacc(
            "TRN2", target_bir_lowering=False, debug=False, num_devices=NCORES
        )
        ins, out = _declare_dram(nc)
        with tile.TileContext(nc) as tc:
            emit_moe(tc, out, ins)
        nc.compile()
        _CACHED = nc
    return _CACHED


def kernel(**inputs) -> np.ndarray:
    from concourse.bass_utils import run_bass_kernel_spmd

    per_core = _prep_host(**inputs)
    nc = _build()
    res = run_bass_kernel_spmd(nc, per_core, core_ids=list(range(NCORES)))
    return np.concatenate([r["out"] for r in res.results], axis=0)


if __name__ == "__main__":
    import reference

    inp = {k: np.asarray(v) for k, v in reference.setup_inputs().items()}
    got = kernel(**inp)
    exp = np.asarray(reference.reference(**inp))
    err = np.abs(got - exp).max() / np.abs(exp).max()
    print("Relative error:", err)


# revision 39
# speedup vs baseline: 1.0765x; 1.0765x over previous
"""MoE routing kernel for Trainium2 (8 NeuronCores, data-parallel over batch).

Reference computation (B=1024, PHASE=64, GATE=128, K=8, D=512):
    coeff = softmax(gateMLP(phase))                       # [B, K]
    per layer l in 0..2:
        y = sum_k coeff[:,k] * (y @ W_l[k]) + coeff @ b_l # [B, D]
        y = elu(y)  (layers 0,1 only)

Device mapping (per core, B_local = 128 rows). The schedule is built
around the measured DMA fabric: 16 engines x ~25 GB/s = ~400 GB/s
aggregate; packets are served FIFO per queue and round-robin BY PACKET
across queues (so small-row blobs starve behind 4KB weight rows), and
the Scalar HWDGE adds ~1.5-2.4us descriptor->packet latency vs ~0.15us
on the Sync HWDGE.  Hence ALL loads go on the Sync queue, consts first:
  - one merged fp16 const blob (gate weights + biases, ph.T packed into
    the unused partitions below gw0) then x.T, then W0[0..7], the bias
    banks, W1, W2; the last expert chunk (2,7) is split per-IC so its
    matmuls start per 128KB sub-chunk.  ~12.9 MB total => ~32us stream.
  - Activations carry a +1 shift: y' = elu(y)+1, shift absorbed into the
    next layer's bias (b' = b - W.sum(axis=in)) on the host.  ELU is
    r=relu(x) on DVE, e=exp(x) on ACT, out=min(e,1)+r on DVE.  Engines
    reading the same PSUM bank serialize, so boundary ELUs are split in
    column halves (reads r0,e0,r1,e1 interleave; z matmuls for the first
    half start while the second half drains).
  - Softmax is normalized right after the gate (e_hat = e/sum(e)); all
    PSUM drains are scale-free.
  - Layer 0 runs gate-independent per-expert matmuls (x.T @ W0[k], paced
    by W-chunk arrival), hand-interleaved with the gate's small matmuls,
    then post-scales with diag(e_hat) + bias matmul.
  - Layers 1-2 pre-scale: z_k.T = y.T * e_hat[:,k] via y_chunk.T @
    diag(e_hat_k), 4 experts' diagonals per N=512 fp16 matmul; then 32
    accumulating matmuls + one bias matmul per layer into one PSUM bank.
    The z transposes share the 4-bank PSUM pool with layer-0's experts
    (free by then) and drain on three engines (DVE/ACT/GpSimd) so the
    2-bank ping-pong never gates the PE.
  - Final drain is split in column halves across ACT/DVE with two output
    DMAs so the tail pipeline overlaps.
"""

import numpy as np

import concourse.mybir as mybir
import concourse.tile as tile
from concourse import bacc
from concourse.masks import make_identity

AFT = mybir.ActivationFunctionType
ALU = mybir.AluOpType
F32 = mybir.dt.float32
F16 = mybir.dt.float16
AX = mybir.AxisListType

B, PHASE, GATE, K, D = 1024, 64, 128, 8, 512
NCORES = 8
BL = B // NCORES          # 128 rows per core
IC = D // 128             # 4 contraction chunks of 128
LW = K * IC * D           # weight columns per layer (16384)


def emit_moe(tc, out_ap, ins):
    """Emit the per-core MoE program. ins is a dict of DRAM APs."""
    nc = tc.nc

    with (
        tc.tile_pool(name="consts", bufs=1) as cpool,
        tc.tile_pool(name="ypool", bufs=2) as ypool,
        tc.tile_pool(name="zpool", bufs=2) as zpool,
        tc.tile_pool(name="tmp", bufs=3) as tpool,
        tc.tile_pool(name="ps_out", bufs=2, space="PSUM") as ps_out,
        tc.tile_pool(name="ps_z", bufs=2, space="PSUM") as ps_z,
        tc.tile_pool(name="ps_exp", bufs=4, space="PSUM") as ps_exp,
    ):
        # ACT warmup: pull the Exp activation table off the critical path.
        t_ones = cpool.tile([1, GATE], F16)
        nc.vector.memset(t_ones, 1.0)
        t_warm = tpool.tile([1, 8], F32, tag="warm")
        nc.scalar.activation(t_warm, t_ones[:, :8], AFT.Exp)

        # identities built on-chip (gpsimd is idle; saves DMA bytes)
        t_ident = cpool.tile([128, 128], F16)
        make_identity(nc, t_ident)
        t_ident32 = cpool.tile([128, 128], F32)
        make_identity(nc, t_ident32)

        # ---- all consts in ONE descriptor, FIRST on the Sync HWDGE queue
        # (few small descriptors serialize the DGE ring ramp) ------------
        # gc [128, 928] f16:
        #   0:128 gw1 | 128:136 gw2 | 136:137 gb1a | row0 137:145 gb2
        #   160:288 rows 0:64 gw0 | 288:416 rows 0:64 ph.T | 416:928 x
        t_gc = cpool.tile([128, 928], F16)
        nc.sync.dma_start(out=t_gc, in_=ins["gc"])
        t_gw1 = t_gc[:, 0:128]
        t_gw2 = t_gc[:, 128:136]
        t_gb2 = t_gc[0:1, 137:145]
        t_gw0 = t_gc[0:PHASE, 160:288]
        t_phT = t_gc[0:PHASE, 288:416]
        t_xT = t_gc[:, 416:928]
        # scalar APs must be f32: cast the f16 gb1 column on-chip
        t_gb1 = cpool.tile([128, 1], F32)
        nc.vector.tensor_copy(out=t_gb1, in_=t_gc[:, 136:137])

        # ---- expert weights on the same queue: bias banks first (needed
        # by the early-interleaved combine bias matmul), then W0 (consumed
        # while streaming), W1, W2; the final chunk (2,7) split per-IC so
        # its matmuls start per 128KB sub-chunk --------------------------
        t_w = cpool.tile([128, 3 * LW], F16)
        t_bias = cpool.tile([K, 3 * D], F16)
        nc.sync.dma_start(out=t_bias, in_=ins["eb"])

        def w_dma(l, k):
            nc.sync.dma_start(
                out=t_w[:, l * LW + k * 2048:l * LW + (k + 1) * 2048],
                in_=ins["W"][l, k],
            )

        for k in range(K):
            w_dma(0, k)
        for k in range(K):
            w_dma(1, k)
        for k in range(K - 1):
            w_dma(2, k)
        for ic in range(IC):
            nc.sync.dma_start(
                out=t_w[:, 2 * LW + (K - 1) * 2048 + ic * 512:
                        2 * LW + (K - 1) * 2048 + (ic + 1) * 512],
                in_=ins["W"][2, K - 1][:, ic * 512:(ic + 1) * 512],
            )

        # ---- gate + layer-0, with PE emission interleaved: layer-0's
        # gate-independent per-expert matmuls (x.T @ W0[k], paced by
        # W-chunk arrival from ~10.5us) weave between the gate's small
        # latency-bound matmuls ------------------------------------------
        t_pe = zpool.tile([128, K * D], F16, tag="z")
        p_es = {}

        def expert_mms(k):
            p_e = ps_exp.tile([128, 512], F32, tag="pexp", name=f"p_e{k}")
            for ic in range(IC):
                nc.tensor.matmul(
                    p_e,
                    lhsT=t_xT[:, ic * 128:(ic + 1) * 128],
                    rhs=t_w[:, k * 2048 + ic * 512:k * 2048 + (ic + 1) * 512],
                    start=(ic == 0),
                    stop=(ic == 3),
                )
            p_es[k] = p_e

        def expert_copy(k):
            dst = t_pe[:, k * 512:(k + 1) * 512]
            if k % 2 == 0:
                nc.vector.tensor_copy(out=dst, in_=p_es[k])
            else:
                nc.scalar.copy(dst, p_es[k])

        p_g = ps_z.tile([128, 512], F32, tag="zps")
        nc.tensor.matmul(p_g[:GATE, :BL], lhsT=t_gw0, rhs=t_phT, start=True, stop=True)
        h1 = tpool.tile([GATE, BL], F16, tag="h")
        _elu1(nc, tpool, h1, p_g[:GATE, :BL], bias=0.0)

        expert_mms(0)
        expert_mms(1)
        expert_copy(0)
        expert_copy(1)

        p_g2 = ps_z.tile([128, 512], F32, tag="zps")
        nc.tensor.matmul(p_g2[:GATE, :BL], lhsT=t_gw1, rhs=h1, start=True, stop=True)
        h2 = tpool.tile([GATE, BL], F16, tag="h")
        _elu1(nc, tpool, h2, p_g2[:GATE, :BL], bias=t_gb1)

        expert_mms(2)
        expert_mms(3)
        expert_copy(2)
        expert_copy(3)

        # logits[b, k] (normal layout; gb2 via ones-row matmul)
        p_lg = ps_z.tile([128, 512], F32, tag="zps")
        nc.tensor.matmul(p_lg[:BL, :K], lhsT=h2, rhs=t_gw2, start=True, stop=False)
        nc.tensor.matmul(p_lg[:BL, :K], lhsT=t_ones, rhs=t_gb2, start=False, stop=True)

        # e_hat = softmax(logits), normalized immediately (cheap [128,8] ops)
        t_nmx = tpool.tile([BL, 1], F32)
        nc.vector.reduce_max(t_nmx, p_lg[:BL, :K], axis=AX.X, negate=True)
        t_e = cpool.tile([BL, K], F32)
        nc.scalar.activation(t_e, p_lg[:BL, :K], AFT.Exp, bias=t_nmx, scale=1.0)
        t_sum = tpool.tile([BL, 1], F32)
        nc.vector.reduce_sum(t_sum, t_e, axis=AX.X)
        t_rcp = tpool.tile([BL, 1], F32)
        nc.vector.reciprocal(t_rcp, t_sum)
        t_eh = cpool.tile([BL, K], F32)
        nc.vector.tensor_scalar_mul(t_eh, t_e, t_rcp)

        # e_hat.T (fp16) for the mixed-bias matmuls
        p_et = ps_z.tile([128, 512], F32, tag="zps")
        nc.tensor.transpose(p_et[:K, :BL], t_eh, t_ident32)
        t_eT = cpool.tile([K, BL], F16)
        nc.scalar.copy(t_eT, p_et[:K, :BL])

        # diag quads: [diag(eh_{4q}) .. diag(eh_{4q+3})], split DVE/ACT
        t_diag = cpool.tile([128, 2 * 512], F16)
        for k in range(K):
            dst = t_diag[:, k * 128:(k + 1) * 128]
            sc = t_eh[:, k:k + 1]
            if k % 2 == 0:
                nc.vector.tensor_scalar_mul(dst, t_ident, sc)
            else:
                nc.scalar.activation(dst, t_ident, AFT.Copy, scale=sc)

        # layer-0 combine, interleaved with the remaining experts so only
        # the last expert's diag matmul trails the last weight chunk:
        # out = sum_k diag(eh_k) @ pe_k + eh @ b0
        p_o = ps_out.tile([BL, D], F32, tag="out")
        nc.tensor.matmul(
            p_o, lhsT=t_eT, rhs=t_bias[:, 0:D], start=True, stop=False
        )

        def combine_mm(k):
            nc.tensor.matmul(
                p_o,
                lhsT=t_diag[:, k * 128:(k + 1) * 128],
                rhs=t_pe[:, k * 512:(k + 1) * 512],
                start=False,
                stop=(k == K - 1),
            )

        expert_mms(4)
        expert_copy(4)
        combine_mm(0)
        combine_mm(1)
        expert_mms(5)
        expert_copy(5)
        combine_mm(2)
        combine_mm(3)
        expert_mms(6)
        expert_copy(6)
        combine_mm(4)
        expert_mms(7)
        expert_copy(7)
        combine_mm(5)
        combine_mm(6)
        combine_mm(7)

        y = ypool.tile([BL, D], F16, tag="y")
        _elu_split(nc, tpool, y, p_o)

        # ---- layers 1,2: pre-scale via diag matmuls (z = transpose+scale),
        # then bias + 32 accumulating matmuls into one PSUM bank.  z PSUM
        # rotates through the 4-bank ps_exp pool; drains alternate DVE/ACT
        # so bank reuse never gates the PE. -------------------------------
        for l in range(1, 3):
            t_zl = zpool.tile([128, K * D], F16, tag="z")
            zi = 0
            for q in range(2):
                for ic in range(IC):
                    p_z = ps_exp.tile([128, 512], F32, tag="pexp")
                    nc.tensor.matmul(
                        p_z,
                        lhsT=y[:, ic * 128:(ic + 1) * 128],
                        rhs=t_diag[:, q * 512:(q + 1) * 512],
                        start=True,
                        stop=True,
                    )
                    dst = t_zl[:, q * 2048 + ic * 512:q * 2048 + (ic + 1) * 512]
                    if zi % 2 == 0:
                        nc.vector.tensor_copy(out=dst, in_=p_z)
                    else:
                        nc.scalar.copy(dst, p_z)
                    zi += 1

            # step B: out = eh @ b_l + sum_k z_k @ W_l[k] (bias first, so the
            # group's last matmul is the last expert and the drain starts
            # right after the last weight chunk is consumed)
            p_o = ps_out.tile([BL, D], F32, tag="out")
            nc.tensor.matmul(
                p_o,
                lhsT=t_eT,
                rhs=t_bias[:, l * D:(l + 1) * D],
                start=True,
                stop=False,
            )
            for k in range(K):
                q, kq = divmod(k, 4)
                for ic in range(IC):
                    nc.tensor.matmul(
                        p_o,
                        lhsT=t_zl[:, q * 2048 + ic * 512 + kq * 128:
                                  q * 2048 + ic * 512 + (kq + 1) * 128],
                        rhs=t_w[:, l * LW + k * 2048 + ic * 512:
                                l * LW + k * 2048 + (ic + 1) * 512],
                        start=False,
                        stop=(k == K - 1 and ic == IC - 1),
                    )

            if l < 2:
                y_next = ypool.tile([BL, D], F16, tag="y")
                _elu_split(nc, tpool, y_next, p_o)
                y = y_next
            else:
                # drain in column halves on two engines, two output DMAs
                t_out = ypool.tile([BL, D], F32, tag="yout")
                nc.scalar.copy(t_out[:, 0:256], p_o[:, 0:256])
                nc.sync.dma_start(out=out_ap[:, 0:256], in_=t_out[:, 0:256])
                nc.vector.tensor_copy(out=t_out[:, 256:512], in_=p_o[:, 256:512])
                nc.sync.dma_start(out=out_ap[:, 256:512], in_=t_out[:, 256:512])


def _elu1(nc, tpool, out, pre, bias):
    """out = elu(x)+1 = relu(x) + min(exp(x), 1); x = pre + bias.
    Small (gate) version: single-shot."""
    shape = [pre.partition_size(), pre.free_size()]
    t_r = tpool.tile(shape, F16, tag="elu_r")
    nc.vector.tensor_scalar(t_r, pre, bias, 0.0, op0=ALU.add, op1=ALU.max)
    t_e = tpool.tile(shape, F32, tag="elu_e")
    nc.scalar.activation(t_e, pre, AFT.Exp, bias=bias, scale=1.0)
    nc.vector.scalar_tensor_tensor(
        out, in0=t_e, scalar=1.0, in1=t_r, op0=ALU.min, op1=ALU.add
    )


def _elu_split(nc, tpool, out, pre):
    """Column-split elu+1 on a [128, 512] PSUM bank.  PSUM readers
    serialize, so reads go r0, e0, r1, e1 (each half-size); the combine
    for the first half runs while the second half is still being read,
    and downstream matmuls on columns 0:256 can start ~1us earlier."""
    H = pre.free_size() // 2
    P = pre.partition_size()
    rs, es = [], []
    for h in range(2):
        cols = slice(h * H, (h + 1) * H)
        t_r = tpool.tile([P, H], F16, tag=f"elu_r{h}")
        nc.vector.tensor_scalar(t_r, pre[:, cols], 0.0, 0.0, op0=ALU.add, op1=ALU.max)
        t_e = tpool.tile([P, H], F32, tag=f"elu_e{h}")
        nc.scalar.activation(t_e, pre[:, cols], AFT.Exp)
        rs.append(t_r)
        es.append(t_e)
    for h in range(2):
        cols = slice(h * H, (h + 1) * H)
        nc.vector.scalar_tensor_tensor(
            out[:, cols], in0=es[h], scalar=1.0, in1=rs[h], op0=ALU.min, op1=ALU.add
        )


def _prep_host(x, phase, gw0, gb0, gw1, gb1, gw2, gb2, W0, b0, W1, b1, W2, b2):
    """Host-side packing. Returns per-core input maps."""
    f32 = np.float32
    f16 = np.float16

    # weights blob: [3, 8, 128, 2048]; [l, k, p, ic*512 + o] = W_l[k, ic*128+p, o]
    W = np.stack([W0, W1, W2]).astype(f32)  # [3, 8, 512, 512]
    Wb = (
        W.reshape(3, K, IC, 128, D)
        .transpose(0, 1, 3, 2, 4)
        .reshape(3, K, 128, IC * D)
        .astype(f16)
    )
    # +1-shift corrections: layer l>0 consumes y'+1, gate layers 1,2 consume h'+1
    b0a = np.asarray(b0, f32)
    b1a = np.asarray(b1, f32) - np.asarray(W1, f32).sum(axis=1)
    b2a = np.asarray(b2, f32) - np.asarray(W2, f32).sum(axis=1)
    eb = np.concatenate([b0a, b1a, b2a], axis=1).astype(f16)  # [8, 1536]
    gb1a = np.asarray(gb1, f32) - np.asarray(gw1, f32).sum(axis=0)
    gb2a = np.asarray(gb2, f32) - np.asarray(gw2, f32).sum(axis=0)

    # gc blob [128, 928] f16 (layout documented in emit_moe)
    gc = np.zeros((128, 928), f16)
    gc[:, 0:128] = np.asarray(gw1, f32).astype(f16)
    gc[:, 128:136] = np.asarray(gw2, f32).astype(f16)
    gc[:, 136] = gb1a.astype(f16)
    gc[0, 137:145] = gb2a.astype(f16)
    gc[0:PHASE, 160:288] = np.asarray(gw0, f32).astype(f16)

    per_core = []
    for c in range(NCORES):
        sl = slice(c * BL, (c + 1) * BL)
        gcc = gc.copy()
        gcc[0:PHASE, 288:416] = np.asarray(phase[sl], f32).T.astype(f16)
        xs = np.asarray(x[sl]).astype(f16)
        gcc[:, 416:928] = (
            xs.T.reshape(IC, 128, BL).transpose(1, 0, 2).reshape(128, IC * BL)
        )
        per_core.append(
            {
                "gc": np.ascontiguousarray(gcc),
                "eb": eb,
                "W": Wb,
            }
        )
    return per_core


def _declare_dram(nc):
    ins = {
        "gc": nc.dram_tensor("gc", [128, 928], F16, kind="ExternalInput").ap(),
        "eb": nc.dram_tensor("eb", [K, 3 * D], F16, kind="ExternalInput").ap(),
        "W": nc.dram_tensor("W", [3, K, 128, IC * D], F16, kind="ExternalInput").ap(),
    }
    out = nc.dram_tensor("out", [BL, D], mybir.dt.float32, kind="ExternalOutput").ap()
    return ins, out


_CACHED = None


def _build():
    global _CACHED
    if _CACHED is None:
        nc = bacc.Bacc(
            "TRN2", target_bir_lowering=False, debug=False, num_devices=NCORES
        )
        ins, out = _declare_dram(nc)
        with tile.TileContext(nc) as tc:
            emit_moe(tc, out, ins)
        nc.compile()
        _CACHED = nc
    return _CACHED


def kernel(**inputs) -> np.ndarray:
    from concourse.bass_utils import run_bass_kernel_spmd

    per_core = _prep_host(**inputs)
    nc = _build()
    res = run_bass_kernel_spmd(nc, per_core, core_ids=list(range(NCORES)))
    return np.concatenate([r["out"] for r in res.results], axis=0)


if __name__ == "__main__":
    import reference

    inp = {k: np.asarray(v) for k, v in reference.setup_inputs().items()}
    got = kernel(**inp)
    exp = np.asarray(reference.reference(**inp))
    err = np.abs(got - exp).max() / np.abs(exp).max()
    print("Relative error:", err)
